# revision 28
# baseline (speedup 1.0000x reference)
"""Trainium2 Bass kernel for nn_AdditiveCouplingLayer — v3: fp8 DoubleRow
matmuls, paired-PSUM drains, odd-only device I/O.

y = x; y[:, 1::2] += MLP(x[:, 0::2])  with a 512->1024->1024->512 relu MLP.

Data-parallel over 8 NeuronCores (batch 65536 -> 8192/core), weights
replicated. The even (conditioning) columns of y are exactly x's even
columns, so the device never sees them: the host sends the masked half
pre-transposed+quantized (mT) and the odd columns (xo = x_odd + b3, fp16),
the device returns yo = xo + MLP(mT), and the host re-interleaves.

The kernel is PE-bound at the fp8 DoubleRow streaming rate (~518 PE
cycles per 512-free-dim MM pass, 1024 passes/core ~ 221us @2.4GHz).
v3 changes vs v2 (244.5us measured):
 - batch tile 1024 split into two 512-column PSUM banks of the SAME
   hidden chunk, drained by ONE paired ACTIVATE [128,2,512] (bias is
   per-partition so the pair is legal). Halves the ACT instruction
   count; ACT (relu+fp8-quant) was 82% busy and paced PSUM recycling.
 - yo stores ride the gpsimd SWDGE queue: a scalar-queue store costs
   the ACT sequencer ~0.7us each and delayed PSUM drains.
 - startup: DVE-side scratch memset + 4 junk warmup MMs (HAM window),
   priority DMA order (W1 pair0 + mT tile0 half0 land first on separate
   HWDGE rings), so real MMs start ~3us earlier.
 - L3 uses paired [128,2,512] PSUM + one DVE add per pair; final tile
   falls back to chunk-granular add+store (alternating sync/scalar
   queues) to keep the post-last-MM tail short.
"""

import sys

sys.path.insert(0, "/opt/trn_rl_repo")

import numpy as np

B, D, F, H = 65536, 1024, 512, 1024
NCORES = 8
BPC = B // NCORES  # rows per core
TB = 1024  # batch rows per tile-iteration
HTB = 512  # matmul free dim = one fp32 PSUM bank
NBT = BPC // TB  # tile-iterations per core
NWARM = 8

_cache = {}


def _build():
    import concourse.bacc as bacc
    import concourse.tile as tile
    import concourse.mybir as mybir

    dt = mybir.dt
    AF = mybir.ActivationFunctionType
    DR = mybir.MatmulPerfMode.DoubleRow
    adt = dt.float8e4

    nc = bacc.Bacc(
        "TRN2", target_bir_lowering=False, debug=False, num_devices=NCORES
    )

    # xo/yo travel as fp16: the residual values are O(1), fp16 rounding
    # adds ~1e-4 to the rel err and halves load+store traffic.
    xo_d = nc.dram_tensor("xo", [BPC, F], dt.float16, kind="ExternalInput").ap()
    # mT is host-pre-tiled to [NBT*128, 4*TB]: row (bt*128 + p) holds the
    # batch-tile-bt slice for all 4 feature k-chunks (4KB contiguous per
    # partition per tile load).
    mT_d = nc.dram_tensor("mT", [NBT * 128, 4 * TB], adt, kind="ExternalInput").ap()
    w_d = {}
    for name, shape in (("w1", [F, H]), ("w2", [H, H]), ("w3", [H, F])):
        w_d[name] = nc.dram_tensor(name, shape, adt, kind="ExternalInput").ap()
    b1_d = nc.dram_tensor("b1m", [128, H // 128], dt.float32, kind="ExternalInput").ap()
    b2_d = nc.dram_tensor("b2m", [128, H // 128], dt.float32, kind="ExternalInput").ap()
    yo_d = nc.dram_tensor("yo", [BPC, F], dt.float16, kind="ExternalOutput").ap()

    with tile.TileContext(nc) as tc:
        with (
            tc.tile_pool(name="wpool", bufs=1) as wpool,
            tc.tile_pool(name="mpool", bufs=3) as mpool,
            tc.tile_pool(name="xpool", bufs=3) as xpool,
            tc.tile_pool(name="hpool", bufs=3) as hpool,
            tc.tile_pool(name="psA", bufs=3, space="PSUM") as psA,
            tc.tile_pool(name="psB", bufs=1, space="PSUM") as psB,
        ):
            # PE warmup: junk matmuls on a zeroed scratch keep the PE
            # busy through its HAM activity window (~3.4us to 2.4GHz)
            # while the first real DMAs are in flight. The junk train must
            # reach the first real MM (~11.7us, DMA-gated) with no PE idle
            # gap, else the HAM busy-window resets and real MMs run cold.
            # scratch doubles as the junk-MM operand and the zeros tensor
            # for DVE relu drains; the first half gates the junk train
            # (fast memset), the second half is only needed ~13us in.
            scratch = wpool.tile([128, 2, HTB], dt.float16, tag="scratch")
            nc.gpsimd.memset(scratch[:, 0:1, :], 0.0)
            nc.gpsimd.memset(scratch[:, 1:2, :], 0.0)
            pwarm = psA.tile([128, 2, HTB], dt.float32, tag="mm", name="pwarm")
            for _ in range(NWARM):
                nc.tensor.matmul(
                    pwarm[:, 0:1, :], scratch[:, 0:1, :128], scratch[:, 0:1, :],
                    start=True, stop=True,
                )

            # --- resident weights/biases. Startup DMA order IS the
            # critical path: the first real MM needs mT tile0 half0
            # (sync ring) + W1 pair0 (scalar ring), so those go first on
            # their rings; W2 is needed ~8us later, W3 ~12us later.
            w1t = wpool.tile([128, 4, H], adt, tag="w1")
            w2t = wpool.tile([128, 8, H], adt, tag="w2")
            w3t = wpool.tile([128, 8, F], adt, tag="w3")
            w1s = w_d["w1"].rearrange("(k p) c -> p k c", p=128)
            w2s = w_d["w2"].rearrange("(k p) c -> p k c", p=128)
            w3s = w_d["w3"].rearrange("(k p) c -> p k c", p=128)

            # b1/b2 + W1 pair0 on the scalar (ACT) ring — only 3 issues
            # (each costs the ACT sequencer ~0.7us), so the first ACTIVATE
            # dispatches early. W2/W3 go on the sync ring BEHIND the mT
            # tile-0/W1-pair1 loads: serialized there they cannot steal
            # HBM bandwidth from the startup-critical transfers (putting
            # them on a third queue made them concurrent and pushed the
            # first real MM from ~12us to ~20us), and the SP sequencer
            # issue cost is free. They still land ~6us before layer 2
            # first needs them.
            # W1 (both pairs) ALONE on the scalar ring: a bias DMA is 128
            # 32-byte descriptors, and two of those ahead of W1p0 in the
            # ring FIFO delay its completion by ~4us (measured). The
            # biases go via gpsimd SWDGE (separate queue rows — no HWDGE
            # ring impact); they land ~10.5us, first ACTIVATE needs them
            # ~13us. W1 pair1 here (not on sync) so both rings deliver
            # the layer-1 pass-1 operands by ~12us in parallel.
            nc.scalar.dma_start(w1t[:, 0:2, :], w1s[:, 0:2, :])
            nc.scalar.dma_start(w1t[:, 2:4, :], w1s[:, 2:4, :])
            b1t = wpool.tile([128, H // 128], dt.float32, tag="b1t")
            nc.gpsimd.dma_start(b1t[:], b1_d[:])
            b2t = wpool.tile([128, H // 128], dt.float32, tag="b2t")
            nc.gpsimd.dma_start(b2t[:], b2_d[:])

            mts = {}

            def load_mT(i, split=False):
                t = mpool.tile([128, 4, 2, HTB], adt, tag="mt", name="mt")
                src = mT_d[i * 128 : (i + 1) * 128, :]
                if split:
                    # tile 0: half0 (k-chunks 0,1) gates the first MM
                    nc.sync.dma_start(t[:, 0:2, :, :], src[:, 0 : 2 * TB])
                    nc.sync.dma_start(t[:, 2:4, :, :], src[:, 2 * TB :])
                else:
                    nc.sync.dma_start(t[:], src[:])
                mts[i] = t

            def load_w23():
                for j in range(4):
                    nc.sync.dma_start(
                        w2t[:, 2 * j : 2 * j + 2, :], w2s[:, 2 * j : 2 * j + 2, :]
                    )
                nc.sync.dma_start(w3t[:, 0:4, :], w3s[:, 0:4, :])
                nc.sync.dma_start(w3t[:, 4:8, :], w3s[:, 4:8, :])

            def load_xo(i):
                t = xpool.tile([128, TB // 128, HTB], dt.float16, tag="xo", name="xo")
                nc.sync.dma_start(
                    t[:], xo_d[i * TB : (i + 1) * TB, :].rearrange("(i p) c -> p i c", p=128)
                )
                return t

            load_mT(0, split=True)
            load_mT(1)
            load_w23()
            xos = {0: load_xo(0)}

            def l1_group(i, h1, m, dve_drain=False):
                """Layer-1 output chunk m for tile i: 2 PSUM banks (batch
                halves) x 2 k-pair passes, one paired relu+quant drain.
                dve_drain: relu on the vector engine instead —
                (psum + b1) max 0 as one scalar_tensor_tensor (bias is a
                legal per-partition scalar; zeros come from scratch). Used
                in the prologue, where the pure-L1 burst is ACT-bound."""
                mT = mts[i]
                p = psA.tile([128, 2, HTB], dt.float32, tag="mm", name="p1")
                ms = slice(m * 128, (m + 1) * 128)
                for b in range(2):
                    for j in range(2):
                        nc.tensor.matmul(
                            p[:, b : b + 1, :],
                            w1t[:, 2 * j : 2 * j + 2, ms],
                            mT[:, 2 * j : 2 * j + 2, b : b + 1, :],
                            start=(j == 0), stop=(j == 1), perf_mode=DR,
                        )
                if dve_drain:
                    nc.vector.scalar_tensor_tensor(
                        h1[:, m : m + 1, :, :], p[:], b1t[:, m : m + 1],
                        scratch[:],
                        op0=mybir.AluOpType.add, op1=mybir.AluOpType.max,
                    )
                else:
                    nc.scalar.activation(
                        h1[:, m : m + 1, :, :], p[:], AF.Relu, bias=b1t[:, m : m + 1]
                    )

            def l2_group(h1, h2, m):
                p = psA.tile([128, 2, HTB], dt.float32, tag="mm", name="p2")
                ms = slice(m * 128, (m + 1) * 128)
                for b in range(2):
                    for j in range(4):
                        nc.tensor.matmul(
                            p[:, b : b + 1, :],
                            w2t[:, 2 * j : 2 * j + 2, ms],
                            h1[:, 2 * j : 2 * j + 2, b : b + 1, :],
                            start=(j == 0), stop=(j == 3), perf_mode=DR,
                        )
                nc.scalar.activation(
                    h2[:, m : m + 1, :, :], p[:], AF.Relu, bias=b2t[:, m : m + 1]
                )

            def new_h(tag):
                return hpool.tile([128, 8, 2, HTB], adt, tag=tag, name=tag)

            h1 = new_h("h1")
            for m in range(8):
                # odd groups drain on DVE: the prologue is ACT-serial-
                # bound (8 x 1.11us drains vs 8 x 0.86us PE fills)
                l1_group(0, h1, m, dve_drain=(m % 2 == 1))

            pending = None
            for i in range(NBT):
                if i + 2 < NBT:
                    load_mT(i + 2)
                if i + 1 < NBT:
                    xos[i + 1] = load_xo(i + 1)
                if pending is not None:
                    # yo store for tile i-1 on the gpsimd SWDGE queue:
                    # keeps the ACT ring free for ACTIVATEs and can't
                    # block the sync ring's mT/xo loads.
                    nc.gpsimd.dma_start(*pending)
                    pending = None

                h2 = new_h("h2")
                h1n = new_h("h1") if i + 1 < NBT else None
                for m in range(8):
                    l2_group(h1, h2, m)
                    if h1n is not None:
                        l1_group(i + 1, h1n, m)

                xot = xos.pop(i)
                last = i == NBT - 1
                for pi in range(4):
                    # final tile: alternate psA/psB so pair pi+2's MMs only
                    # wait on pair pi's drain (a 1-buf pool fully
                    # serializes MM->add chains on the last tile)
                    pool = (psA if pi % 2 else psB) if last else psB
                    p3 = pool.tile([128, 2, HTB], dt.float32, tag="mm" if (last and pi % 2) else "m3", name="p3")
                    chunked = last or (i == NBT - 2 and pi == 3)
                    if chunked:
                        # chunk-granular drains: interleave the two bank
                        # groups (j-outer) so bank ci=1's group START
                        # isn't serialized by Tile against the pending
                        # add reading bank 0 of the same tile (measured
                        # ~1.7us of waits when issued group-major).
                        for j in range(4):
                            for ci in range(2):
                                ch = 2 * pi + ci
                                bh, cs = ch // 4, (ch % 4) * 128
                                nc.tensor.matmul(
                                    p3[:, ci : ci + 1, :],
                                    h2[:, 2 * j : 2 * j + 2, bh : bh + 1, cs : cs + 128],
                                    w3t[:, 2 * j : 2 * j + 2, :],
                                    start=(j == 0), stop=(j == 3), perf_mode=DR,
                                    skip_group_check=True,
                                )
                    for ci in range(2):
                        ch = 2 * pi + ci
                        bh, cs = ch // 4, (ch % 4) * 128
                        if not chunked:
                            for j in range(4):
                                nc.tensor.matmul(
                                    p3[:, ci : ci + 1, :],
                                    h2[:, 2 * j : 2 * j + 2, bh : bh + 1, cs : cs + 128],
                                    w3t[:, 2 * j : 2 * j + 2, :],
                                    start=(j == 0), stop=(j == 3), perf_mode=DR,
                                )
                        if last:
                            # final tile: chunk-granular add+store with
                            # alternating queues so the tail after the
                            # last MM stays short (last store is 128KB).
                            xi = xot[:, ch : ch + 1, :]
                            nc.vector.tensor_add(xi, xi, p3[:, ci : ci + 1, :])
                            eng = nc.sync if ch % 2 == 0 else nc.scalar
                            eng.dma_start(
                                yo_d[i * TB + ch * 128 : i * TB + (ch + 1) * 128, :],
                                xi,
                            )
                        elif chunked:
                            # second-to-last tile, last pair: chunk adds
                            # (0.7us each) instead of one 1.24us paired
                            # add, so the final tile's first chunk add
                            # doesn't queue behind it in the DVE FIFO.
                            xi = xot[:, ch : ch + 1, :]
                            nc.vector.tensor_add(xi, xi, p3[:, ci : ci + 1, :])
                    if not chunked:
                        xp = xot[:, 2 * pi : 2 * pi + 2, :]
                        nc.vector.tensor_add(xp, xp, p3[:])
                if not last:
                    pending = (
                        yo_d[i * TB : (i + 1) * TB, :].rearrange("(i p) c -> p i c", p=128),
                        xot[:],
                    )
                if h1n is not None:
                    h1 = h1n

    nc.compile()
    return nc


# test.py compatibility: MODE is vestigial (fp8 only).
MODE = "fp8"


def _get(mode="fp8"):
    if "nc" not in _cache:
        _cache["nc"] = _build()
    return _cache["nc"]


def _in_maps(x, W1, b1, W2, b2, W3, b3):
    import ml_dtypes

    qdt = ml_dtypes.float8_e4m3

    ws = {
        name: np.asarray(w, np.float32).astype(qdt)
        for name, w in (("w1", W1), ("w2", W2), ("w3", W3))
    }

    common = dict(
        ws,
        # host pre-transposes biases to [128, n/128] so the DMA is contiguous
        b1m=np.ascontiguousarray(np.asarray(b1, np.float32).reshape(-1, 128).T),
        b2m=np.ascontiguousarray(np.asarray(b2, np.float32).reshape(-1, 128).T),
    )
    x = np.asarray(x, np.float32)
    b3f = np.asarray(b3, np.float32)
    in_maps = []
    for c in range(NCORES):
        xs = x[c * BPC : (c + 1) * BPC]
        masked_t = xs[:, 0::2].T.astype(qdt)  # [F, BPC] fp8
        # pre-tile to [NBT*128, 4*TB]: row (bt*128+p) = all 4 k-chunks of
        # batch-tile bt, so each device tile load is fully contiguous
        mt = np.ascontiguousarray(
            masked_t.reshape(4, 128, NBT, TB).transpose(2, 1, 0, 3)
        ).reshape(NBT * 128, 4 * TB)
        in_maps.append(
            dict(
                common,
                # b3 is folded into the residual here (one fused pass)
                # so the device never does the bias pre-add
                xo=(xs[:, 1::2] + b3f).astype(np.float16),
                mT=mt,
            )
        )
    return in_maps


def kernel(x, W1, b1, W2, b2, W3, b3):
    from concourse.bass_utils import run_bass_kernel_spmd

    nc = _get()
    x = np.asarray(x, np.float32)
    res = run_bass_kernel_spmd(
        nc, _in_maps(x, W1, b1, W2, b2, W3, b3), core_ids=list(range(NCORES))
    )
    y = np.empty((B, D), dtype=np.float32)
    y[:, 0::2] = x[:, 0::2]
    yo = np.concatenate([res.results[c]["yo"] for c in range(NCORES)], axis=0)
    y[:, 1::2] = yo.astype(np.float32)
    return y


# revision 34
# speedup vs baseline: 1.0066x; 1.0066x over previous
"""Trainium2 Bass kernel for nn_AdditiveCouplingLayer — v3: fp8 DoubleRow
matmuls, paired-PSUM drains, odd-only device I/O.

y = x; y[:, 1::2] += MLP(x[:, 0::2])  with a 512->1024->1024->512 relu MLP.

Data-parallel over 8 NeuronCores (batch 65536 -> 8192/core), weights
replicated. The even (conditioning) columns of y are exactly x's even
columns, so the device never sees them: the host sends the masked half
pre-transposed+quantized (mT) and the odd columns (xo = x_odd + b3, fp16),
the device returns yo = xo + MLP(mT), and the host re-interleaves.

The kernel is PE-bound at the fp8 DoubleRow streaming rate (~518 PE
cycles per 512-free-dim MM pass, 1024 passes/core ~ 221us @2.4GHz).
v3 changes vs v2 (244.5us measured):
 - batch tile 1024 split into two 512-column PSUM banks of the SAME
   hidden chunk, drained by ONE paired ACTIVATE [128,2,512] (bias is
   per-partition so the pair is legal). Halves the ACT instruction
   count; ACT (relu+fp8-quant) was 82% busy and paced PSUM recycling.
 - yo stores ride the gpsimd SWDGE queue: a scalar-queue store costs
   the ACT sequencer ~0.7us each and delayed PSUM drains.
 - startup: DVE-side scratch memset + 4 junk warmup MMs (HAM window),
   priority DMA order (W1 pair0 + mT tile0 half0 land first on separate
   HWDGE rings), so real MMs start ~3us earlier.
 - L3 uses paired [128,2,512] PSUM + one DVE add per pair; final tile
   falls back to chunk-granular add+store (alternating sync/scalar
   queues) to keep the post-last-MM tail short.
"""

import sys

sys.path.insert(0, "/opt/trn_rl_repo")

import numpy as np

B, D, F, H = 65536, 1024, 512, 1024
NCORES = 8
BPC = B // NCORES  # rows per core
TB = 1024  # batch rows per tile-iteration
HTB = 512  # matmul free dim = one fp32 PSUM bank
NBT = BPC // TB  # tile-iterations per core
NWARM = 8

_cache = {}


def _build():
    import concourse.bacc as bacc
    import concourse.tile as tile
    import concourse.mybir as mybir

    dt = mybir.dt
    AF = mybir.ActivationFunctionType
    DR = mybir.MatmulPerfMode.DoubleRow
    adt = dt.float8e4

    nc = bacc.Bacc(
        "TRN2", target_bir_lowering=False, debug=False, num_devices=NCORES
    )

    # xo/yo travel as fp16: the residual values are O(1), fp16 rounding
    # adds ~1e-4 to the rel err and halves load+store traffic.
    xo_d = nc.dram_tensor("xo", [BPC, F], dt.float16, kind="ExternalInput").ap()
    # mT is host-pre-tiled to [NBT*128, 4*TB]: row (bt*128 + p) holds the
    # batch-tile-bt slice for all 4 feature k-chunks (4KB contiguous per
    # partition per tile load).
    mT_d = nc.dram_tensor("mT", [NBT * 128, 4 * TB], adt, kind="ExternalInput").ap()
    w_d = {}
    for name, shape in (("w1", [F, H]), ("w2", [H, H]), ("w3", [H, F])):
        w_d[name] = nc.dram_tensor(name, shape, adt, kind="ExternalInput").ap()
    b1_d = nc.dram_tensor("b1m", [128, H // 128], dt.float32, kind="ExternalInput").ap()
    b2_d = nc.dram_tensor("b2m", [128, H // 128], dt.float32, kind="ExternalInput").ap()
    yo_d = nc.dram_tensor("yo", [BPC, F], dt.float16, kind="ExternalOutput").ap()

    with tile.TileContext(nc) as tc:
        with (
            tc.tile_pool(name="wpool", bufs=1) as wpool,
            tc.tile_pool(name="mpool", bufs=3) as mpool,
            tc.tile_pool(name="xpool", bufs=3) as xpool,
            tc.tile_pool(name="hpool", bufs=3) as hpool,
            tc.tile_pool(name="psA", bufs=3, space="PSUM") as psA,
            tc.tile_pool(name="psB", bufs=1, space="PSUM") as psB,
        ):
            # PE warmup: junk matmuls on a zeroed scratch keep the PE
            # busy through its HAM activity window (~3.4us to 2.4GHz)
            # while the first real DMAs are in flight. The junk train must
            # reach the first real MM (~11.7us, DMA-gated) with no PE idle
            # gap, else the HAM busy-window resets and real MMs run cold.
            scratch = wpool.tile([128, HTB], dt.float16, tag="scratch")
            nc.gpsimd.memset(scratch[:], 0.0)
            pwarm = psA.tile([128, 2, HTB], dt.float32, tag="mm", name="pwarm")
            for _ in range(NWARM):
                nc.tensor.matmul(
                    pwarm[:, 0:1, :], scratch[:, :128], scratch[:],
                    start=True, stop=True,
                )

            # --- resident weights/biases. Startup DMA order IS the
            # critical path: the first real MM needs mT tile0 half0
            # (sync ring) + W1 pair0 (scalar ring), so those go first on
            # their rings; W2 is needed ~8us later, W3 ~12us later.
            w1t = wpool.tile([128, 4, H], adt, tag="w1")
            w2t = wpool.tile([128, 8, H], adt, tag="w2")
            w3t = wpool.tile([128, 8, F], adt, tag="w3")
            w1s = w_d["w1"].rearrange("(k p) c -> p k c", p=128)
            w2s = w_d["w2"].rearrange("(k p) c -> p k c", p=128)
            w3s = w_d["w3"].rearrange("(k p) c -> p k c", p=128)

            # b1/b2 + W1 pair0 on the scalar (ACT) ring — only 3 issues
            # (each costs the ACT sequencer ~0.7us), so the first ACTIVATE
            # dispatches early. W2/W3 go on the sync ring BEHIND the mT
            # tile-0/W1-pair1 loads: serialized there they cannot steal
            # HBM bandwidth from the startup-critical transfers (putting
            # them on a third queue made them concurrent and pushed the
            # first real MM from ~12us to ~20us), and the SP sequencer
            # issue cost is free. They still land ~6us before layer 2
            # first needs them.
            # W1 (both pairs) ALONE on the scalar ring: a bias DMA is 128
            # 32-byte descriptors, and two of those ahead of W1p0 in the
            # ring FIFO delay its completion by ~4us (measured). The
            # biases go via gpsimd SWDGE (separate queue rows — no HWDGE
            # ring impact); they land ~10.5us, first ACTIVATE needs them
            # ~13us. W1 pair1 here (not on sync) so both rings deliver
            # the layer-1 pass-1 operands by ~12us in parallel.
            nc.scalar.dma_start(w1t[:, 0:2, :], w1s[:, 0:2, :])
            nc.scalar.dma_start(w1t[:, 2:4, :], w1s[:, 2:4, :])
            b1t = wpool.tile([128, H // 128], dt.float32, tag="b1t")
            nc.gpsimd.dma_start(b1t[:], b1_d[:])
            b2t = wpool.tile([128, H // 128], dt.float32, tag="b2t")
            nc.gpsimd.dma_start(b2t[:], b2_d[:])

            mts = {}

            def load_mT(i, split=False):
                t = mpool.tile([128, 4, 2, HTB], adt, tag="mt", name="mt")
                src = mT_d[i * 128 : (i + 1) * 128, :]
                if split:
                    # tile 0: half0 (k-chunks 0,1) gates the first MM
                    nc.sync.dma_start(t[:, 0:2, :, :], src[:, 0 : 2 * TB])
                    nc.sync.dma_start(t[:, 2:4, :, :], src[:, 2 * TB :])
                else:
                    nc.sync.dma_start(t[:], src[:])
                mts[i] = t

            def load_w23():
                for j in range(4):
                    nc.sync.dma_start(
                        w2t[:, 2 * j : 2 * j + 2, :], w2s[:, 2 * j : 2 * j + 2, :]
                    )
                nc.sync.dma_start(w3t[:, 0:4, :], w3s[:, 0:4, :])
                nc.sync.dma_start(w3t[:, 4:8, :], w3s[:, 4:8, :])

            def load_xo(i):
                t = xpool.tile([128, TB // 128, HTB], dt.float16, tag="xo", name="xo")
                nc.sync.dma_start(
                    t[:], xo_d[i * TB : (i + 1) * TB, :].rearrange("(i p) c -> p i c", p=128)
                )
                return t

            load_mT(0, split=True)
            load_mT(1)
            load_w23()
            xos = {0: load_xo(0)}

            def l1_group(i, h1, m):
                """Layer-1 output chunk m for tile i: 2 PSUM banks (batch
                halves) x 2 k-pair passes, one paired relu+quant drain."""
                mT = mts[i]
                p = psA.tile([128, 2, HTB], dt.float32, tag="mm", name="p1")
                ms = slice(m * 128, (m + 1) * 128)
                for b in range(2):
                    for j in range(2):
                        nc.tensor.matmul(
                            p[:, b : b + 1, :],
                            w1t[:, 2 * j : 2 * j + 2, ms],
                            mT[:, 2 * j : 2 * j + 2, b : b + 1, :],
                            start=(j == 0), stop=(j == 1), perf_mode=DR,
                        )
                nc.scalar.activation(
                    h1[:, m : m + 1, :, :], p[:], AF.Relu, bias=b1t[:, m : m + 1]
                )

            def l2_group(h1, h2, m):
                p = psA.tile([128, 2, HTB], dt.float32, tag="mm", name="p2")
                ms = slice(m * 128, (m + 1) * 128)
                for b in range(2):
                    for j in range(4):
                        nc.tensor.matmul(
                            p[:, b : b + 1, :],
                            w2t[:, 2 * j : 2 * j + 2, ms],
                            h1[:, 2 * j : 2 * j + 2, b : b + 1, :],
                            start=(j == 0), stop=(j == 3), perf_mode=DR,
                        )
                nc.scalar.activation(
                    h2[:, m : m + 1, :, :], p[:], AF.Relu, bias=b2t[:, m : m + 1]
                )

            def new_h(tag):
                return hpool.tile([128, 8, 2, HTB], adt, tag=tag, name=tag)

            h1 = new_h("h1")
            for m in range(8):
                l1_group(0, h1, m)

            pending = None
            for i in range(NBT):
                if i + 2 < NBT:
                    load_mT(i + 2)
                if i + 1 < NBT:
                    xos[i + 1] = load_xo(i + 1)
                if pending is not None:
                    # yo store for tile i-1 on the gpsimd SWDGE queue:
                    # keeps the ACT ring free for ACTIVATEs and can't
                    # block the sync ring's mT/xo loads.
                    nc.gpsimd.dma_start(*pending)
                    pending = None

                h2 = new_h("h2")
                h1n = new_h("h1") if i + 1 < NBT else None
                for m in range(8):
                    l2_group(h1, h2, m)
                    if h1n is not None:
                        l1_group(i + 1, h1n, m)

                xot = xos.pop(i)
                last = i == NBT - 1
                for pi in range(4):
                    # final tile: alternate psA/psB so pair pi+2's MMs only
                    # wait on pair pi's drain (a 1-buf pool fully
                    # serializes MM->add chains on the last tile)
                    pool = (psA if pi % 2 else psB) if last else psB
                    p3 = pool.tile([128, 2, HTB], dt.float32, tag="mm" if (last and pi % 2) else "m3", name="p3")
                    for ci in range(2):
                        ch = 2 * pi + ci
                        bh, cs = ch // 4, (ch % 4) * 128
                        for j in range(4):
                            nc.tensor.matmul(
                                p3[:, ci : ci + 1, :],
                                h2[:, 2 * j : 2 * j + 2, bh : bh + 1, cs : cs + 128],
                                w3t[:, 2 * j : 2 * j + 2, :],
                                start=(j == 0), stop=(j == 3), perf_mode=DR,
                            )
                        if last:
                            # final tile: chunk-granular add+store with
                            # alternating queues so the tail after the
                            # last MM stays short (last store is 128KB).
                            xi = xot[:, ch : ch + 1, :]
                            nc.vector.tensor_add(xi, xi, p3[:, ci : ci + 1, :])
                            eng = nc.sync if ch % 2 == 0 else nc.scalar
                            eng.dma_start(
                                yo_d[i * TB + ch * 128 : i * TB + (ch + 1) * 128, :],
                                xi,
                            )
                        elif i == NBT - 2 and pi == 3:
                            # second-to-last tile, last pair: chunk adds
                            # (0.7us each) instead of one 1.24us paired
                            # add, so the final tile's first chunk add
                            # doesn't queue behind it in the DVE FIFO.
                            xi = xot[:, ch : ch + 1, :]
                            nc.vector.tensor_add(xi, xi, p3[:, ci : ci + 1, :])
                    if not last and not (i == NBT - 2 and pi == 3):
                        xp = xot[:, 2 * pi : 2 * pi + 2, :]
                        nc.vector.tensor_add(xp, xp, p3[:])
                if not last:
                    pending = (
                        yo_d[i * TB : (i + 1) * TB, :].rearrange("(i p) c -> p i c", p=128),
                        xot[:],
                    )
                if h1n is not None:
                    h1 = h1n

    nc.compile()
    return nc


# test.py compatibility: MODE is vestigial (fp8 only).
MODE = "fp8"


def _get(mode="fp8"):
    if "nc" not in _cache:
        _cache["nc"] = _build()
    return _cache["nc"]


def _in_maps(x, W1, b1, W2, b2, W3, b3):
    import ml_dtypes

    qdt = ml_dtypes.float8_e4m3

    ws = {
        name: np.asarray(w, np.float32).astype(qdt)
        for name, w in (("w1", W1), ("w2", W2), ("w3", W3))
    }

    common = dict(
        ws,
        # host pre-transposes biases to [128, n/128] so the DMA is contiguous
        b1m=np.ascontiguousarray(np.asarray(b1, np.float32).reshape(-1, 128).T),
        b2m=np.ascontiguousarray(np.asarray(b2, np.float32).reshape(-1, 128).T),
    )
    x = np.asarray(x, np.float32)
    b3f = np.asarray(b3, np.float32)
    in_maps = []
    for c in range(NCORES):
        xs = x[c * BPC : (c + 1) * BPC]
        masked_t = xs[:, 0::2].T.astype(qdt)  # [F, BPC] fp8
        # pre-tile to [NBT*128, 4*TB]: row (bt*128+p) = all 4 k-chunks of
        # batch-tile bt, so each device tile load is fully contiguous
        mt = np.ascontiguousarray(
            masked_t.reshape(4, 128, NBT, TB).transpose(2, 1, 0, 3)
        ).reshape(NBT * 128, 4 * TB)
        in_maps.append(
            dict(
                common,
                # b3 is folded into the residual here (one fused pass)
                # so the device never does the bias pre-add
                xo=(xs[:, 1::2] + b3f).astype(np.float16),
                mT=mt,
            )
        )
    return in_maps


def kernel(x, W1, b1, W2, b2, W3, b3):
    from concourse.bass_utils import run_bass_kernel_spmd

    nc = _get()
    x = np.asarray(x, np.float32)
    res = run_bass_kernel_spmd(
        nc, _in_maps(x, W1, b1, W2, b2, W3, b3), core_ids=list(range(NCORES))
    )
    y = np.empty((B, D), dtype=np.float32)
    y[:, 0::2] = x[:, 0::2]
    yo = np.concatenate([res.results[c]["yo"] for c in range(NCORES)], axis=0)
    y[:, 1::2] = yo.astype(np.float32)
    return y


# revision 35
# speedup vs baseline: 1.0103x; 1.0036x over previous
"""Trainium2 Bass kernel for nn_AdditiveCouplingLayer — v3: fp8 DoubleRow
matmuls, paired-PSUM drains, odd-only device I/O.

y = x; y[:, 1::2] += MLP(x[:, 0::2])  with a 512->1024->1024->512 relu MLP.

Data-parallel over 8 NeuronCores (batch 65536 -> 8192/core), weights
replicated. The even (conditioning) columns of y are exactly x's even
columns, so the device never sees them: the host sends the masked half
pre-transposed+quantized (mT) and the odd columns (xo = x_odd + b3, fp16),
the device returns yo = xo + MLP(mT), and the host re-interleaves.

The kernel is PE-bound at the fp8 DoubleRow streaming rate (~518 PE
cycles per 512-free-dim MM pass, 1024 passes/core ~ 221us @2.4GHz).
v3 changes vs v2 (244.5us measured):
 - batch tile 1024 split into two 512-column PSUM banks of the SAME
   hidden chunk, drained by ONE paired ACTIVATE [128,2,512] (bias is
   per-partition so the pair is legal). Halves the ACT instruction
   count; ACT (relu+fp8-quant) was 82% busy and paced PSUM recycling.
 - yo stores ride the gpsimd SWDGE queue: a scalar-queue store costs
   the ACT sequencer ~0.7us each and delayed PSUM drains.
 - startup: DVE-side scratch memset + 4 junk warmup MMs (HAM window),
   priority DMA order (W1 pair0 + mT tile0 half0 land first on separate
   HWDGE rings), so real MMs start ~3us earlier.
 - L3 uses paired [128,2,512] PSUM + one DVE add per pair; final tile
   falls back to chunk-granular add+store (alternating sync/scalar
   queues) to keep the post-last-MM tail short.
"""

import sys

sys.path.insert(0, "/opt/trn_rl_repo")

import numpy as np

B, D, F, H = 65536, 1024, 512, 1024
NCORES = 8
BPC = B // NCORES  # rows per core
TB = 1024  # batch rows per tile-iteration
HTB = 512  # matmul free dim = one fp32 PSUM bank
NBT = BPC // TB  # tile-iterations per core
NWARM = 8

_cache = {}


def _build():
    import concourse.bacc as bacc
    import concourse.tile as tile
    import concourse.mybir as mybir

    dt = mybir.dt
    AF = mybir.ActivationFunctionType
    DR = mybir.MatmulPerfMode.DoubleRow
    adt = dt.float8e4

    nc = bacc.Bacc(
        "TRN2", target_bir_lowering=False, debug=False, num_devices=NCORES
    )

    # xo/yo travel as fp16: the residual values are O(1), fp16 rounding
    # adds ~1e-4 to the rel err and halves load+store traffic.
    xo_d = nc.dram_tensor("xo", [BPC, F], dt.float16, kind="ExternalInput").ap()
    # mT is host-pre-tiled to [NBT*128, 4*TB]: row (bt*128 + p) holds the
    # batch-tile-bt slice for all 4 feature k-chunks (4KB contiguous per
    # partition per tile load).
    mT_d = nc.dram_tensor("mT", [NBT * 128, 4 * TB], adt, kind="ExternalInput").ap()
    w_d = {}
    for name, shape in (("w1", [F, H]), ("w2", [H, H]), ("w3", [H, F])):
        w_d[name] = nc.dram_tensor(name, shape, adt, kind="ExternalInput").ap()
    b1_d = nc.dram_tensor("b1m", [128, H // 128], dt.float32, kind="ExternalInput").ap()
    b2_d = nc.dram_tensor("b2m", [128, H // 128], dt.float32, kind="ExternalInput").ap()
    yo_d = nc.dram_tensor("yo", [BPC, F], dt.float16, kind="ExternalOutput").ap()

    with tile.TileContext(nc) as tc:
        with (
            tc.tile_pool(name="wpool", bufs=1) as wpool,
            tc.tile_pool(name="mpool", bufs=3) as mpool,
            tc.tile_pool(name="xpool", bufs=3) as xpool,
            tc.tile_pool(name="hpool", bufs=3) as hpool,
            tc.tile_pool(name="psA", bufs=3, space="PSUM") as psA,
            tc.tile_pool(name="psB", bufs=1, space="PSUM") as psB,
        ):
            # PE warmup: junk matmuls on a zeroed scratch keep the PE
            # busy through its HAM activity window (~3.4us to 2.4GHz)
            # while the first real DMAs are in flight. The junk train must
            # reach the first real MM (~11.7us, DMA-gated) with no PE idle
            # gap, else the HAM busy-window resets and real MMs run cold.
            scratch = wpool.tile([128, HTB], dt.float16, tag="scratch")
            nc.gpsimd.memset(scratch[:], 0.0)
            pwarm = psA.tile([128, 2, HTB], dt.float32, tag="mm", name="pwarm")
            for _ in range(NWARM):
                nc.tensor.matmul(
                    pwarm[:, 0:1, :], scratch[:, :128], scratch[:],
                    start=True, stop=True,
                )

            # --- resident weights/biases. Startup DMA order IS the
            # critical path: the first real MM needs mT tile0 half0
            # (sync ring) + W1 pair0 (scalar ring), so those go first on
            # their rings; W2 is needed ~8us later, W3 ~12us later.
            w1t = wpool.tile([128, 4, H], adt, tag="w1")
            w2t = wpool.tile([128, 8, H], adt, tag="w2")
            w3t = wpool.tile([128, 8, F], adt, tag="w3")
            w1s = w_d["w1"].rearrange("(k p) c -> p k c", p=128)
            w2s = w_d["w2"].rearrange("(k p) c -> p k c", p=128)
            w3s = w_d["w3"].rearrange("(k p) c -> p k c", p=128)

            # b1/b2 + W1 pair0 on the scalar (ACT) ring — only 3 issues
            # (each costs the ACT sequencer ~0.7us), so the first ACTIVATE
            # dispatches early. W2/W3 go on the sync ring BEHIND the mT
            # tile-0/W1-pair1 loads: serialized there they cannot steal
            # HBM bandwidth from the startup-critical transfers (putting
            # them on a third queue made them concurrent and pushed the
            # first real MM from ~12us to ~20us), and the SP sequencer
            # issue cost is free. They still land ~6us before layer 2
            # first needs them.
            # W1 (both pairs) ALONE on the scalar ring: a bias DMA is 128
            # 32-byte descriptors, and two of those ahead of W1p0 in the
            # ring FIFO delay its completion by ~4us (measured). The
            # biases go via gpsimd SWDGE (separate queue rows — no HWDGE
            # ring impact); they land ~10.5us, first ACTIVATE needs them
            # ~13us. W1 pair1 here (not on sync) so both rings deliver
            # the layer-1 pass-1 operands by ~12us in parallel.
            nc.scalar.dma_start(w1t[:, 0:2, :], w1s[:, 0:2, :])
            nc.scalar.dma_start(w1t[:, 2:4, :], w1s[:, 2:4, :])
            b1t = wpool.tile([128, H // 128], dt.float32, tag="b1t")
            nc.gpsimd.dma_start(b1t[:], b1_d[:])
            b2t = wpool.tile([128, H // 128], dt.float32, tag="b2t")
            nc.gpsimd.dma_start(b2t[:], b2_d[:])

            mts = {}

            def load_mT(i, split=False):
                t = mpool.tile([128, 4, 2, HTB], adt, tag="mt", name="mt")
                src = mT_d[i * 128 : (i + 1) * 128, :]
                if split:
                    # tile 0: half0 (k-chunks 0,1) gates the first MM
                    nc.sync.dma_start(t[:, 0:2, :, :], src[:, 0 : 2 * TB])
                    nc.sync.dma_start(t[:, 2:4, :, :], src[:, 2 * TB :])
                else:
                    nc.sync.dma_start(t[:], src[:])
                mts[i] = t

            def load_w23():
                for j in range(4):
                    nc.sync.dma_start(
                        w2t[:, 2 * j : 2 * j + 2, :], w2s[:, 2 * j : 2 * j + 2, :]
                    )
                nc.sync.dma_start(w3t[:, 0:4, :], w3s[:, 0:4, :])
                nc.sync.dma_start(w3t[:, 4:8, :], w3s[:, 4:8, :])

            def load_xo(i):
                t = xpool.tile([128, TB // 128, HTB], dt.float16, tag="xo", name="xo")
                nc.sync.dma_start(
                    t[:], xo_d[i * TB : (i + 1) * TB, :].rearrange("(i p) c -> p i c", p=128)
                )
                return t

            load_mT(0, split=True)
            load_mT(1)
            load_w23()
            xos = {0: load_xo(0)}

            def l1_group(i, h1, m):
                """Layer-1 output chunk m for tile i: 2 PSUM banks (batch
                halves) x 2 k-pair passes, one paired relu+quant drain."""
                mT = mts[i]
                p = psA.tile([128, 2, HTB], dt.float32, tag="mm", name="p1")
                ms = slice(m * 128, (m + 1) * 128)
                for b in range(2):
                    for j in range(2):
                        nc.tensor.matmul(
                            p[:, b : b + 1, :],
                            w1t[:, 2 * j : 2 * j + 2, ms],
                            mT[:, 2 * j : 2 * j + 2, b : b + 1, :],
                            start=(j == 0), stop=(j == 1), perf_mode=DR,
                        )
                nc.scalar.activation(
                    h1[:, m : m + 1, :, :], p[:], AF.Relu, bias=b1t[:, m : m + 1]
                )

            def l2_group(h1, h2, m):
                p = psA.tile([128, 2, HTB], dt.float32, tag="mm", name="p2")
                ms = slice(m * 128, (m + 1) * 128)
                for b in range(2):
                    for j in range(4):
                        nc.tensor.matmul(
                            p[:, b : b + 1, :],
                            w2t[:, 2 * j : 2 * j + 2, ms],
                            h1[:, 2 * j : 2 * j + 2, b : b + 1, :],
                            start=(j == 0), stop=(j == 3), perf_mode=DR,
                        )
                nc.scalar.activation(
                    h2[:, m : m + 1, :, :], p[:], AF.Relu, bias=b2t[:, m : m + 1]
                )

            def new_h(tag):
                return hpool.tile([128, 8, 2, HTB], adt, tag=tag, name=tag)

            h1 = new_h("h1")
            for m in range(8):
                l1_group(0, h1, m)

            pending = None
            for i in range(NBT):
                if i + 2 < NBT:
                    load_mT(i + 2)
                if i + 1 < NBT:
                    xos[i + 1] = load_xo(i + 1)
                if pending is not None:
                    # yo store for tile i-1 on the gpsimd SWDGE queue:
                    # keeps the ACT ring free for ACTIVATEs and can't
                    # block the sync ring's mT/xo loads.
                    nc.gpsimd.dma_start(*pending)
                    pending = None

                h2 = new_h("h2")
                h1n = new_h("h1") if i + 1 < NBT else None
                for m in range(8):
                    l2_group(h1, h2, m)
                    if h1n is not None:
                        l1_group(i + 1, h1n, m)

                xot = xos.pop(i)
                last = i == NBT - 1
                for pi in range(4):
                    # final tile: alternate psA/psB so pair pi+2's MMs only
                    # wait on pair pi's drain (a 1-buf pool fully
                    # serializes MM->add chains on the last tile)
                    pool = (psA if pi % 2 else psB) if last else psB
                    p3 = pool.tile([128, 2, HTB], dt.float32, tag="mm" if (last and pi % 2) else "m3", name="p3")
                    if last and pi % 2 == 0:
                        # psB pairs of the final tile: issue the two bank
                        # groups' MMs interleaved (j-outer) so bank 1's
                        # group START isn't serialized by Tile against
                        # the pending chunk-add reading bank 0 of the
                        # same psum tile (the two ~0.7-1.2us stalls
                        # measured here in group-major order).
                        for j in range(4):
                            for ci in range(2):
                                ch = 2 * pi + ci
                                bh, cs = ch // 4, (ch % 4) * 128
                                nc.tensor.matmul(
                                    p3[:, ci : ci + 1, :],
                                    h2[:, 2 * j : 2 * j + 2, bh : bh + 1, cs : cs + 128],
                                    w3t[:, 2 * j : 2 * j + 2, :],
                                    start=(j == 0), stop=(j == 3), perf_mode=DR,
                                    skip_group_check=True,
                                )
                    for ci in range(2):
                        ch = 2 * pi + ci
                        bh, cs = ch // 4, (ch % 4) * 128
                        if not (last and pi % 2 == 0):
                            for j in range(4):
                                nc.tensor.matmul(
                                    p3[:, ci : ci + 1, :],
                                    h2[:, 2 * j : 2 * j + 2, bh : bh + 1, cs : cs + 128],
                                    w3t[:, 2 * j : 2 * j + 2, :],
                                    start=(j == 0), stop=(j == 3), perf_mode=DR,
                                )
                        if last:
                            # final tile: chunk-granular add+store with
                            # alternating queues so the tail after the
                            # last MM stays short (last store is 128KB).
                            xi = xot[:, ch : ch + 1, :]
                            nc.vector.tensor_add(xi, xi, p3[:, ci : ci + 1, :])
                            eng = nc.sync if ch % 2 == 0 else nc.scalar
                            eng.dma_start(
                                yo_d[i * TB + ch * 128 : i * TB + (ch + 1) * 128, :],
                                xi,
                            )
                        elif i == NBT - 2 and pi == 3:
                            # second-to-last tile, last pair: chunk adds
                            # (0.7us each) instead of one 1.24us paired
                            # add, so the final tile's first chunk add
                            # doesn't queue behind it in the DVE FIFO.
                            xi = xot[:, ch : ch + 1, :]
                            nc.vector.tensor_add(xi, xi, p3[:, ci : ci + 1, :])
                    if not last and not (i == NBT - 2 and pi == 3):
                        xp = xot[:, 2 * pi : 2 * pi + 2, :]
                        nc.vector.tensor_add(xp, xp, p3[:])
                if not last:
                    pending = (
                        yo_d[i * TB : (i + 1) * TB, :].rearrange("(i p) c -> p i c", p=128),
                        xot[:],
                    )
                if h1n is not None:
                    h1 = h1n

    nc.compile()
    return nc


# test.py compatibility: MODE is vestigial (fp8 only).
MODE = "fp8"


def _get(mode="fp8"):
    if "nc" not in _cache:
        _cache["nc"] = _build()
    return _cache["nc"]


def _in_maps(x, W1, b1, W2, b2, W3, b3):
    import ml_dtypes

    qdt = ml_dtypes.float8_e4m3

    ws = {
        name: np.asarray(w, np.float32).astype(qdt)
        for name, w in (("w1", W1), ("w2", W2), ("w3", W3))
    }

    common = dict(
        ws,
        # host pre-transposes biases to [128, n/128] so the DMA is contiguous
        b1m=np.ascontiguousarray(np.asarray(b1, np.float32).reshape(-1, 128).T),
        b2m=np.ascontiguousarray(np.asarray(b2, np.float32).reshape(-1, 128).T),
    )
    x = np.asarray(x, np.float32)
    b3f = np.asarray(b3, np.float32)
    in_maps = []
    for c in range(NCORES):
        xs = x[c * BPC : (c + 1) * BPC]
        masked_t = xs[:, 0::2].T.astype(qdt)  # [F, BPC] fp8
        # pre-tile to [NBT*128, 4*TB]: row (bt*128+p) = all 4 k-chunks of
        # batch-tile bt, so each device tile load is fully contiguous
        mt = np.ascontiguousarray(
            masked_t.reshape(4, 128, NBT, TB).transpose(2, 1, 0, 3)
        ).reshape(NBT * 128, 4 * TB)
        in_maps.append(
            dict(
                common,
                # b3 is folded into the residual here (one fused pass)
                # so the device never does the bias pre-add
                xo=(xs[:, 1::2] + b3f).astype(np.float16),
                mT=mt,
            )
        )
    return in_maps


def kernel(x, W1, b1, W2, b2, W3, b3):
    from concourse.bass_utils import run_bass_kernel_spmd

    nc = _get()
    x = np.asarray(x, np.float32)
    res = run_bass_kernel_spmd(
        nc, _in_maps(x, W1, b1, W2, b2, W3, b3), core_ids=list(range(NCORES))
    )
    y = np.empty((B, D), dtype=np.float32)
    y[:, 0::2] = x[:, 0::2]
    yo = np.concatenate([res.results[c]["yo"] for c in range(NCORES)], axis=0)
    y[:, 1::2] = yo.astype(np.float32)
    return y
